# revision 17
# baseline (speedup 1.0000x reference)
"""Cross-attention kernel for Trainium2, 8-way SPMD (head-sharded).

Problem: B=2, Lt=Ls=2048, D=1024, H=16 heads x 64 dim.
  out = softmax(x@Wq (mem@Wk)^T/8 + pos + mask) @ (mem@Wv) @ Wo

Sharding: 16 heads / 8 cores = 2 heads per core, both batches on every
core (position_embedding is broadcast over batch, so each pos element is
read exactly once system-wide). After attention, an AllToAll re-shards
from head-split context to t-row-split, and each core computes its 512
rows of the output projection with the full Wo.

Device numerics: fp16 matmul operands, fp32 PSUM accumulation.
exp(S+pos+mask) is computed as exp(S)*exp(pos+mask-4) (the -4 shift
cancels in the softmax normalization and keeps fp16 in range). For a
fraction of s-tiles the raw (pos+mask-4) is instead added into the
scores PSUM by an identity matmul on the TensorE, which keeps the PE
busy enough to hold its HAM clock-gate at full speed and offloads the
DVE multiply.
"""
import sys
import numpy as np
from contextlib import ExitStack

for _p in ("/opt/trn_rl_repo",):
    if _p not in sys.path:
        sys.path.append(_p)

import concourse.bacc as bacc
import concourse.tile as tile
from concourse import mybir
from concourse.masks import make_identity
from concourse.bass_utils import run_bass_kernel_spmd

F16 = mybir.dt.float16
F32 = mybir.dt.float32

NCORES = 8
B = 2
LT = 2048
LS = 2048
D = 1024
H = 16
HD = 64
HPC = H // NCORES          # heads per core = 2
TB = 512                   # t block
NTB = LT // TB             # 4 t blocks per batch
ST = 128                   # s tile
NST = LS // ST             # 16 s tiles
KC = 128                   # contraction chunk
NKC = D // KC              # 8 chunks
ROWS = B * LT              # 4096 flattened rows
RPC = ROWS // NCORES       # 512 output rows per core

TRACE = False
LAST_EXEC_NS = None
_CACHE = {}

# s-tiles with st % INJECT_EVERY == 0 carry raw (pos+mask-4) and get added
# into the scores PSUM via an identity matmul on the PE; the rest carry
# exp(pos+mask-4) and are multiplied in on the DVE after the exp.
INJECT_EVERY = 2
N_HEAT_START = 35          # PE warm-up matmuls at kernel start
N_HEAT_A2A = 0            # paced PE keep-warm chain across the AllToAll


def _is_inject(st):
    return st % INJECT_EVERY == 0


def _build_program():
    nc = bacc.Bacc("TRN2", target_bir_lowering=False, debug=False,
                   num_devices=NCORES)

    # ---- DRAM I/O ----
    # pre-transposed activations, blocked [b, blk, 128p, 8k, 512]
    xT = nc.dram_tensor("xT", [B, NTB, 128, NKC, TB], F16, kind="ExternalInput").ap()
    mT = nc.dram_tensor("mT", [B, NTB, 128, NKC, TB], F16, kind="ExternalInput").ap()
    # weights pre-arranged [128p, 8k, cols]
    wq = nc.dram_tensor("wq", [128, NKC, 128], F16, kind="ExternalInput").ap()
    wk = nc.dram_tensor("wk", [128, NKC, 128], F16, kind="ExternalInput").ap()
    wv = nc.dram_tensor("wv", [128, NKC, 128], F16, kind="ExternalInput").ap()
    wo = nc.dram_tensor("wo", [128, NKC, D], F16, kind="ExternalInput").ap()
    # pos+mask tiles, transposed to [s,t]: blocked [tb, h, st, 128s, 512t]
    epm = nc.dram_tensor("epm", [NTB, HPC, NST, ST, TB], F16,
                         kind="ExternalInput").ap()
    out = nc.dram_tensor("out", [RPC, D], F32, kind="ExternalOutput").ap()

    # ctx blocks, already transposed: [shard j][ts][128 cols][128 t]
    ctx_dram = nc.dram_tensor("ctx_dram", [NCORES, 4, 128, 128], F16)
    cat_dram = nc.dram_tensor("cat_dram", [NCORES, 4, 128, 128], F16)

    with tile.TileContext(nc) as tc, ExitStack() as ctx:
        persist = ctx.enter_context(tc.tile_pool(name="persist", bufs=1))

        wq_sb = persist.tile([128, NKC, 128], F16, tag="wq")
        wk_sb = persist.tile([128, NKC, 128], F16, tag="wk")
        wv_sb = persist.tile([128, NKC, 128], F16, tag="wv")
        wo_sb = persist.tile([128, NKC, D], F16, tag="wo")
        nc.sync.dma_start(out=wq_sb, in_=wq)
        nc.sync.dma_start(out=wk_sb, in_=wk)
        nc.sync.dma_start(out=wv_sb, in_=wv)
        nc.sync.dma_start(out=wo_sb, in_=wo)

        ident16 = persist.tile([128, 128], F16, tag="id16")
        make_identity(nc, ident16)
        ident32 = persist.tile([128, 128], F32, tag="id32")
        make_identity(nc, ident32)

        qT_sb = persist.tile([128, B, LT], F16, tag="qT")
        kT_sb = persist.tile([128, B, LS], F16, tag="kT")
        # v augmented with a ones column per head: [s, v_h0 | 1 | v_h1 | 1]
        vaug_sb = persist.tile([128, B, NST, 130], F16, tag="vaug")
        nc.vector.memset(vaug_sb, 1.0)

        # start-of-kernel PE heater: warm the HAM clock gate while the
        # first DMAs are in flight.
        heat_a = persist.tile([128, 512], F16, tag="heat_a")
        nc.vector.memset(heat_a, 0.001)
        with tc.tile_pool(name="heat_ps0", bufs=1, space="PSUM") as hp0:
            hps = hp0.tile([128, 512], F32, tag="hps")
            for _ in range(N_HEAT_START):
                nc.tensor.matmul(hps, lhsT=heat_a[:, 0:128], rhs=heat_a,
                                 start=True, stop=True, skip_group_check=True)

        # ---------------- Phase 1: projections ----------------
        with ExitStack() as p1:
            act_in = p1.enter_context(tc.tile_pool(name="act_in", bufs=3))
            projps = p1.enter_context(
                tc.tile_pool(name="projps", bufs=2, space="PSUM"))

            for b in range(B):
                for blk in range(NTB):
                    xt = act_in.tile([128, NKC, TB], F16, tag="xT")
                    nc.sync.dma_start(out=xt, in_=xT[b, blk])
                    qps = projps.tile([128, TB], F32, tag="qps")
                    for k in range(NKC):
                        nc.tensor.matmul(qps, lhsT=wq_sb[:, k, :],
                                         rhs=xt[:, k, :],
                                         start=(k == 0), stop=(k == NKC - 1))
                    nc.scalar.copy(qT_sb[:, b, blk * TB:(blk + 1) * TB], qps)

                    mt = act_in.tile([128, NKC, TB], F16, tag="mT")
                    nc.sync.dma_start(out=mt, in_=mT[b, blk])
                    kps = projps.tile([128, TB], F32, tag="qps")
                    for k in range(NKC):
                        nc.tensor.matmul(kps, lhsT=wk_sb[:, k, :],
                                         rhs=mt[:, k, :],
                                         start=(k == 0), stop=(k == NKC - 1))
                    nc.scalar.copy(kT_sb[:, b, blk * TB:(blk + 1) * TB], kps)

                    for ssub in range(4):
                        vps = projps.tile([128, 128], F32, tag="vps")
                        for k in range(NKC):
                            nc.tensor.matmul(
                                vps,
                                lhsT=mt[:, k, ssub * 128:(ssub + 1) * 128],
                                rhs=wv_sb[:, k, :],
                                start=(k == 0), stop=(k == NKC - 1))
                        sch = blk * 4 + ssub
                        nc.vector.tensor_copy(vaug_sb[:, b, sch, 0:64],
                                              vps[:, 0:64])
                        nc.vector.tensor_copy(vaug_sb[:, b, sch, 65:129],
                                              vps[:, 64:128])

        # ---------------- Phase 2: attention ----------------
        heat_b = persist.tile([128, 128], F16, tag="heat_b")
        with ExitStack() as p2:
            spool = p2.enter_context(
                tc.tile_pool(name="spool", bufs=2, space="PSUM"))
            ctxps = p2.enter_context(
                tc.tile_pool(name="ctxps", bufs=4, space="PSUM"))
            em_pool = p2.enter_context(tc.tile_pool(name="em_pool", bufs=6))
            e_pool = p2.enter_context(tc.tile_pool(name="e_pool", bufs=6))
            pp_pool = p2.enter_context(tc.tile_pool(name="pp_pool", bufs=4))
            cl_pool = p2.enter_context(tc.tile_pool(name="cl_pool", bufs=2))
            cn_pool = p2.enter_context(tc.tile_pool(name="cn_pool", bufs=3))
            cnt_pool = p2.enter_context(tc.tile_pool(name="cnt_pool", bufs=3))
            rl_pool = p2.enter_context(tc.tile_pool(name="rl_pool", bufs=4))

            for tb in range(NTB):
                ctxL = {}
                for bb in range(B):
                    for h in range(HPC):
                        ctxL[(bb, h)] = ctxps.tile(
                            [65, TB], F32, tag="ctxL",
                            name=f"ctxL_{tb}_{bb}_{h}")
                for st in range(NST):
                    inject = _is_inject(st)
                    for h in range(HPC):
                        ep = em_pool.tile([ST, TB], F16, tag="em",
                                          name=f"em_{tb}_{st}_{h}")
                        nc.gpsimd.dma_start(out=ep, in_=epm[tb, h, st])
                        # S holds both batches side by side: [:, bb*512...]
                        s_ps = spool.tile([ST, 2 * TB], F32, tag="S")
                        for bb in range(B):
                            half = s_ps[:, bb * TB:(bb + 1) * TB]
                            nc.tensor.matmul(
                                half,
                                lhsT=kT_sb[64 * h:64 * (h + 1), bb,
                                           st * ST:(st + 1) * ST],
                                rhs=qT_sb[64 * h:64 * (h + 1), bb,
                                          tb * TB:(tb + 1) * TB],
                                start=True, stop=not inject,
                                skip_group_check=True)
                            if inject:
                                nc.tensor.matmul(
                                    half, lhsT=ident16, rhs=ep,
                                    start=False, stop=True,
                                    skip_group_check=True)
                        e_sb = e_pool.tile([ST, 2 * TB], F16, tag="E")
                        nc.scalar.activation(
                            e_sb, s_ps, mybir.ActivationFunctionType.Exp)
                        for bb in range(B):
                            if inject:
                                pv_rhs = e_sb[:, bb * TB:(bb + 1) * TB]
                            else:
                                p_sb = pp_pool.tile([ST, TB], F16, tag="P")
                                nc.vector.tensor_mul(
                                    p_sb, e_sb[:, bb * TB:(bb + 1) * TB], ep)
                                pv_rhs = p_sb
                            nc.tensor.matmul(
                                ctxL[(bb, h)],
                                lhsT=vaug_sb[:, bb, st, 65 * h:65 * (h + 1)],
                                rhs=pv_rhs,
                                start=(st == 0), stop=(st == NST - 1),
                                skip_group_check=True)
                # epilogue: normalize, transpose, store ctx^T blocks
                for bb in range(B):
                    cl = {}
                    for h in range(HPC):
                        cl[h] = cl_pool.tile([65, TB], F32, tag="cl",
                                             name=f"cl_{tb}_{bb}_{h}")
                        nc.vector.tensor_copy(cl[h], ctxL[(bb, h)])
                    shard = bb * NTB + tb
                    for ts in range(4):
                        cn = cn_pool.tile([128, 128], F16, tag="cn")
                        for h in range(HPC):
                            cps = ctxps.tile([128, 65], F32, tag="ctxL",
                                             name=f"cps_{tb}_{bb}_{ts}_{h}")
                            nc.tensor.transpose(
                                cps, cl[h][:, ts * 128:(ts + 1) * 128],
                                ident32[0:65, 0:65])
                            rl = rl_pool.tile([128, 1], F32, tag="rl")
                            nc.vector.reciprocal(rl, cps[:, 64:65])
                            nc.vector.tensor_scalar_mul(
                                cn[:, 64 * h:64 * (h + 1)], cps[:, 0:64], rl)
                        nc.sync.dma_start(out=ctx_dram.ap()[shard, ts],
                                          in_=cn)
                        if tb == NTB - 1 and bb == B - 1 and ts == 3:
                            # seed for the A2A keep-warm chain: gates the
                            # chain start on the tail of phase 2
                            nc.vector.tensor_copy(heat_b, cn)

        # ---------------- Phase 3: AllToAll + output projection ----------
        nc.gpsimd.collective_compute(
            "AllToAll", mybir.AluOpType.bypass,
            replica_groups=[list(range(NCORES))],
            ins=[ctx_dram.ap()], outs=[cat_dram.ap()])

        # paced PE keep-warm chain across the collective: each matmul is
        # gated by a DVE copy of the previous result, so the chain trickles
        # ~0.5-1us per link instead of bursting.
        with tc.tile_pool(name="heat_ps1", bufs=2, space="PSUM") as hp1:
            for i in range(N_HEAT_A2A):
                hps1 = hp1.tile([128, 128], F32, tag="hps1",
                                name=f"heat1_{i}")
                nc.tensor.matmul(hps1, lhsT=heat_b, rhs=heat_b,
                                 start=True, stop=True,
                                 skip_group_check=True)
                nc.vector.tensor_copy(heat_b, hps1)

        with ExitStack() as p3:
            catT = persist.tile([128, NCORES, RPC], F16, tag="catT")
            ops_ps = p3.enter_context(
                tc.tile_pool(name="ops_ps", bufs=2, space="PSUM"))
            o_pool = p3.enter_context(tc.tile_pool(name="o_pool", bufs=2))

            for j in range(NCORES):
                for ts in range(4):
                    nc.sync.dma_start_transpose(
                        out=catT[:, j, ts * 128:(ts + 1) * 128],
                        in_=cat_dram.ap()[j, ts])
            for ts in range(4):
                for nh in range(2):
                    ops = ops_ps.tile([128, 512], F32, tag="ops")
                    for j in range(NCORES):
                        nc.tensor.matmul(
                            ops,
                            lhsT=catT[:, j, ts * 128:(ts + 1) * 128],
                            rhs=wo_sb[:, j, nh * 512:(nh + 1) * 512],
                            start=(j == 0), stop=(j == NCORES - 1))
                    osb = o_pool.tile([128, 512], F32, tag="osb")
                    nc.scalar.copy(osb, ops)
                    nc.sync.dma_start(
                        out=out[ts * 128:(ts + 1) * 128,
                                nh * 512:(nh + 1) * 512],
                        in_=osb)

    nc.compile()
    return nc


def _prep_inputs(x, memory, position_embedding, mask, Wq, Wk, Wv, Wo):
    """Host-side shard + relayout. Returns per-core input maps."""
    xf = np.asarray(x, np.float32).reshape(ROWS, D)
    mf = np.asarray(memory, np.float32).reshape(ROWS, D)

    def block_T(a):
        # [4096, 1024] -> transpose -> [2, 4, 128, 8, 512] fp16
        at = np.ascontiguousarray(a.T.astype(np.float16))      # [1024, 4096]
        # index [k*128+p, b*2048+blk*512+t]
        v = at.reshape(NKC, KC, B, NTB, TB)
        return np.ascontiguousarray(v.transpose(2, 3, 1, 0, 4))

    xT_b = block_T(xf)
    mT_b = block_T(mf)

    def warr(w, scale=1.0):
        wf = (np.asarray(w, np.float32) * scale).astype(np.float16)
        return np.ascontiguousarray(
            wf.reshape(NKC, KC, wf.shape[1]).transpose(1, 0, 2))

    wo_b = warr(Wo)
    pos = np.asarray(position_embedding, np.float32)[0]        # [16, 2048, 2048]
    maskf = np.asarray(mask, np.float32)

    in_maps = []
    for c in range(NCORES):
        cols = slice(128 * c, 128 * (c + 1))
        wq_b = warr(np.asarray(Wq, np.float32)[:, cols], scale=1.0 / np.sqrt(HD))
        wk_b = warr(np.asarray(Wk, np.float32)[:, cols])
        wv_b = warr(np.asarray(Wv, np.float32)[:, cols])
        eh = np.empty((NTB, HPC, NST, ST, TB), np.float16)
        for i in range(HPC):
            h = HPC * c + i
            pm = (pos[h] + maskf - 4.0).T                       # [s, t]
            # [s, t] -> [st, ST, tb, TB] -> [tb, st, ST, TB]
            blocked = pm.reshape(NST, ST, NTB, TB).transpose(2, 0, 1, 3)
            for st in range(NST):
                if _is_inject(st):
                    eh[:, i, st] = blocked[:, st].astype(np.float16)
                else:
                    eh[:, i, st] = np.exp(blocked[:, st]).astype(np.float16)
        in_maps.append({
            "xT": xT_b, "mT": mT_b, "wq": wq_b, "wk": wk_b, "wv": wv_b,
            "wo": wo_b, "epm": eh,
        })
    return in_maps


def kernel(**inputs):
    global LAST_EXEC_NS
    if "nc" not in _CACHE:
        _CACHE["nc"] = _build_program()
    nc = _CACHE["nc"]
    in_maps = _prep_inputs(**inputs)
    res = run_bass_kernel_spmd(nc, in_maps, list(range(NCORES)), trace=TRACE)
    LAST_EXEC_NS = res.exec_time_ns
    full = np.concatenate([res.results[c]["out"] for c in range(NCORES)],
                          axis=0)
    return full.reshape(B, LT, D)
